# revision 1
# baseline (speedup 1.0000x reference)
"""Discretized-mixture NLL loss kernel for Trainium2 (Bass/Tile), 8-core data parallel.

Math (per pixel, per channel c, mixtures m=0..9), matching the reference:
    xhat = W @ px + b            (1x1 conv, 90 outputs = [pi(30) | mu(30) | ls(30)])
    sigma = exp(8*tanh(ls/8));  s2 = (1/sigma)/sqrt(2) = exp(-8*tanh(ls/8) + ln(1/sqrt2))
    D = mu - xe                  (xe folded into the conv via an extra K=4 matmul)
    dcdf = 0.5*(erf((D+d)*s2) - erf((D-d)*s2))          [erf odd]
    num  = sum_m exp(l_m)*dcdf_m ;  den = sum_m exp(l_m)
    nll  = log(den) - log(num + 1e-8*den)

v2 layout (per core, 16384 px = 16 supertiles of 1024, 2 subtiles of 512 each).
Compute-engine APs must start at partition 0/32/64/96; logical 30-row blocks are
padded to 32.
  - psum tile [128, 1024] (2 banks, pool bufs=3): rows 0..31 D, 32..63 ls,
    64..95 pi, 96..127 tanh-out. fp32r matmuls (full rate when PE warm),
    4 K-chunks of 128 + K=4 chunk carrying (-xe, bias). The psum tile is
    released at the end of phase 1 (mixture reduction uses its own psum pool),
    so the PE pipeline is never serialized behind phase 2.
  - ScalarE: tanh; merged Exp (per-partition scale/bias AP) -> [g | s2].
    Erf over 8-supertile groups (4x [128, 1024] ops) to bound table switches
    (exp-set phase / erf-set phase per group, ~2 ACT_TABLE_LOADs per group).
  - VectorE: fused STT (D -+ delta)*s2 -> hi/lo rows 32*(s%4) of group tiles;
    dlt = Elo - Ehi on GpSimd; qd = g*dlt overwrites dead s2 rows.
  - PE reduction: per subtile one [K=64, M=32, N=512] fp32r matmul into a
    shared psum2 bank ([128, 512]; block base 64*(s%2)+32*t, M=32 cols =
    [n0 n1 n2 X d0 d1 d2 X | 24 dummies], dummies keep Ln inputs finite).
  - ScalarE copy psum2 bank -> scratch; per-subtile respread DMA -> packed
    (row q = 4*v + g', v = L-column 0..15, g' = col/128). Tail per half:
    Ln[64, 2048]; subtract rows 32..63 - 0..31; DMA rows 0..11 out.
"""

import numpy as np

WIDTH = 512
C_IMG = 3
N_MIX = 10
SIZE = 64
STD = 127.5
EPS = 1e-8
DELTA = 1.0 / STD / 2.0
LOG_INV_SQRT2 = -0.34657359027997264
N_CORES = 8
SUP_W = 1024          # pixels per supertile
SUB_W = 512           # matmul moving-dim tile
GRP = 8               # supertiles per activation-table group


def make_consts(W, b):
    """Host-side prep of the small constant tensors (32-padded blocks)."""
    W = np.asarray(W, np.float32)
    b = np.asarray(b, np.float32)
    # lhsT column blocks: [mu(30)+2 | ls(30)+2 | pi(30)+2]; intra-block row 3m+c
    Wp = np.zeros((96, WIDTH), np.float32)
    bp = np.zeros(96, np.float32)
    Wp[0:30], bp[0:30] = W[30:60], b[30:60]     # mu
    Wp[32:62], bp[32:62] = W[60:90], b[60:90]   # logsigma
    Wp[64:94], bp[64:94] = W[0:30], b[0:30]     # pi logits
    wt = np.ascontiguousarray(Wp.T)             # [512, 96]
    bx = np.zeros((4, 96), np.float32)          # K=4 rows: (xe0, xe1, xe2, ones)
    for r in range(30):
        bx[r % 3, r] = -1.0                     # D rows get -xe_c
    bx[3, :] = bp                               # ones row carries the conv bias
    # reduction over s2g = [g(32) | qd(32)]; M=32 columns:
    # v=0..2 num_c, v=3..7 dummy, v=8..10 den_c, v=11..31 dummy (positive)
    l1 = np.zeros((64, 64), np.float32)
    for r in range(30):
        c = r % 3
        l1[r, c] = EPS                          # eps*den folded into num column
        l1[r, 8 + c] = 1.0                      # den
        l1[32 + r, c] = 0.5                     # +0.5*qd -> num
    for v in list(range(3, 8)) + list(range(11, 64)):
        l1[0:30, v] = 1.0                       # dummies = den-like, keep Ln finite
    scb = np.zeros((64, 2), np.float32)         # merged-exp (scale, bias) rows
    scb[0:32, 0] = 1.0                          # pi rows: exp(x)
    scb[32:64, 0] = -8.0                        # tanh rows: exp(-8*t + ln(1/sqrt2))
    scb[32:64, 1] = LOG_INV_SQRT2
    return wt, bx, l1, scb


def build_nc(n_batch=4, use_f32r=True):
    """Build the single-core Bass program (same NEFF runs SPMD on all cores)."""
    from contextlib import ExitStack

    import concourse.bacc as bacc
    import concourse.mybir as mybir
    import concourse.tile as tile
    from concourse.tile import add_dep_helper

    f32 = mybir.dt.float32
    f32r = mybir.dt.float32r
    ALU = mybir.AluOpType
    ACT = mybir.ActivationFunctionType

    def mm_cast(ap):
        return ap.bitcast(f32r) if use_f32r else ap

    sup_per_batch = (SIZE * SIZE) // SUP_W
    S = n_batch * sup_per_batch                 # supertiles per core
    n_sub = S * (SUP_W // SUB_W)                # total subtiles
    grp = min(GRP, S)
    assert S % grp == 0 and grp % 2 == 0

    nc = bacc.Bacc("TRN2", target_bir_lowering=False, debug=False)
    pz = nc.dram_tensor("pz", [n_batch, WIDTH, SIZE * SIZE], f32, kind="ExternalInput").ap()
    x4 = nc.dram_tensor("x4", [S, 4, SUP_W], f32, kind="ExternalInput").ap()
    wt = nc.dram_tensor("wt", [WIDTH, 96], f32, kind="ExternalInput").ap()
    bx = nc.dram_tensor("bx", [4, 96], f32, kind="ExternalInput").ap()
    l1 = nc.dram_tensor("l1", [64, 64], f32, kind="ExternalInput").ap()
    scb = nc.dram_tensor("scb", [64, 2], f32, kind="ExternalInput").ap()
    out = nc.dram_tensor("out", [12, 128 * n_sub], f32, kind="ExternalOutput").ap()

    with tile.TileContext(nc) as tc, ExitStack() as ctx:
        const_pool = ctx.enter_context(tc.tile_pool(name="const", bufs=1))
        xq_pool = ctx.enter_context(tc.tile_pool(name="xq", bufs=2))
        xt_pool = ctx.enter_context(tc.tile_pool(name="xt", bufs=5))
        s2g_pool = ctx.enter_context(tc.tile_pool(name="s2g", bufs=grp + 1))
        hl_pool = ctx.enter_context(tc.tile_pool(name="hl", bufs=2))
        e_pool = ctx.enter_context(tc.tile_pool(name="e", bufs=2))
        dlt_pool = ctx.enter_context(tc.tile_pool(name="dlt", bufs=2))
        sc_pool = ctx.enter_context(tc.tile_pool(name="sc", bufs=2))
        tail_pool = ctx.enter_context(tc.tile_pool(name="tail", bufs=1))
        ps_pool = ctx.enter_context(tc.tile_pool(name="ps", bufs=2, space="PSUM"))
        ps2_pool = ctx.enter_context(tc.tile_pool(name="ps2", bufs=2, space="PSUM"))

        # --- constants ---
        wt_sb = const_pool.tile([128, 4 * 96], f32)
        nc.sync.dma_start(
            wt_sb[:].rearrange("i (k o) -> i k o", o=96).bitcast(f32r),
            wt.rearrange("(k i) o -> i k o", i=128).bitcast(f32r),
        )
        bx_sb = const_pool.tile([4, 96], f32)
        nc.sync.dma_start(bx_sb[:].bitcast(f32r), bx.bitcast(f32r))
        l1_sb = const_pool.tile([64, 64], f32)
        nc.sync.dma_start(l1_sb[:].bitcast(f32r), l1.bitcast(f32r))
        scb_sb = const_pool.tile([64, 2], f32)
        nc.sync.dma_start(scb_sb[:], scb)

        n_half = n_sub // 2
        packed = [tail_pool.tile([64, 128 * n_half], f32, tag=f"packed{i}", name=f"packed{i}")
                  for i in range(2)]

        # ACT table-set ordering chain (tanh/exp/erf/ln; Copy is in every set)
        act_chain = []

        def chain(inst):
            if act_chain:
                add_dep_helper(inst.ins, act_chain[-1].ins, sync=False,
                               reason="act table-set batching")
            act_chain.append(inst)
            return inst

        def phase1(s, hi_t, lo_t):
            b, h = divmod(s, sup_per_batch)
            hb = 32 * ((s % grp) % 4)
            xq_t = xq_pool.tile([4, SUP_W], f32, tag="xq")
            nc.sync.dma_start(xq_t[:].bitcast(f32r), x4[s].bitcast(f32r))
            xts = []
            for k in range(4):
                xt_t = xt_pool.tile([128, SUP_W], f32, tag="xt")
                nc.sync.dma_start(
                    xt_t[:].bitcast(f32r),
                    pz[b, 128 * k:128 * (k + 1), SUP_W * h:SUP_W * (h + 1)].bitcast(f32r),
                )
                xts.append(xt_t)
            ps = ps_pool.tile([128, SUP_W], f32, tag="ps")
            n_t = SUP_W // SUB_W
            for k in range(4):
                for t in range(n_t):
                    sl = slice(SUB_W * t, SUB_W * (t + 1))
                    nc.tensor.matmul(
                        ps[0:96, sl], mm_cast(wt_sb[:, 96 * k:96 * (k + 1)]),
                        mm_cast(xts[k][:, sl]), start=(k == 0), stop=False,
                    )
            for t in range(n_t):
                sl = slice(SUB_W * t, SUB_W * (t + 1))
                nc.tensor.matmul(ps[0:96, sl], mm_cast(bx_sb[:]),
                                 mm_cast(xq_t[:, sl]), start=False, stop=True)
            s2g_t = s2g_pool.tile([64, SUP_W], f32, tag="s2g")
            chain(nc.scalar.activation(ps[96:128, :], ps[32:64, :], ACT.Tanh, scale=0.125))
            chain(nc.scalar.activation(
                s2g_t[:].bitcast(f32r), ps[64:128, :], ACT.Exp,
                bias=scb_sb[:, 1:2], scale=scb_sb[:, 0:1],
            ))
            # hi' = (D - delta)*s2 ; lo' = (D + delta)*s2
            nc.vector.scalar_tensor_tensor(
                hi_t[hb:hb + 32, :], ps[0:32, :], DELTA,
                s2g_t[32:64, :], ALU.subtract, ALU.mult,
            )
            nc.vector.scalar_tensor_tensor(
                lo_t[hb:hb + 32, :], ps[0:32, :], DELTA,
                s2g_t[32:64, :], ALU.add, ALU.mult,
            )
            return s2g_t

        def phase2(s, s2g_t, ehi_t, elo_t, ps2):
            hb = 32 * ((s % grp) % 4)
            g = s2g_t[0:32, :]
            dlt_t = dlt_pool.tile([32, SUP_W], f32, tag="dlt")
            nc.gpsimd.tensor_tensor(dlt_t[:], elo_t[hb:hb + 32, :],
                                    ehi_t[hb:hb + 32, :], ALU.subtract)
            # qd = g*dlt overwrites the dead s2 rows -> [g | qd] contiguous
            nc.vector.tensor_tensor(s2g_t[32:64, :].bitcast(f32r), g, dlt_t[:], ALU.mult)
            for t in range(SUP_W // SUB_W):
                sl = slice(SUB_W * t, SUB_W * (t + 1))
                nc.tensor.matmul(ps2[0:64, sl], mm_cast(l1_sb[:]),
                                 mm_cast(s2g_t[:, sl]), start=True, stop=True)

        def drain_ps2(s, ps2):
            # copy the psum2 banks, then respread each subtile block to packed
            sc_t = sc_pool.tile([64, SUP_W], f32, tag="sc")
            nc.scalar.copy(sc_t[:], ps2[:])
            for t in range(SUP_W // SUB_W):
                sub = s * (SUP_W // SUB_W) + t
                half, subh = divmod(sub, n_half)
                nc.sync.dma_start(
                    packed[half][:, 128 * subh:128 * (subh + 1)],
                    sc_t[0:16, SUB_W * t:SUB_W * (t + 1)]
                    .rearrange("v (g p) -> v g p", p=128),
                )

        n_grp = S // grp
        for gi in range(n_grp):
            hi_ts = [hl_pool.tile([128, SUP_W], f32, tag=f"hi{q}", name=f"hi{q}")
                     for q in range(grp // 4)]
            lo_ts = [hl_pool.tile([128, SUP_W], f32, tag=f"lo{q}", name=f"lo{q}")
                     for q in range(grp // 4)]
            s2gs = []
            for j in range(grp):
                s = gi * grp + j
                s2gs.append(phase1(s, hi_ts[j // 4], lo_ts[j // 4]))
            ehi_ts = [e_pool.tile([128, SUP_W], f32, tag=f"ehi{q}", name=f"ehi{q}")
                      for q in range(grp // 4)]
            elo_ts = [e_pool.tile([128, SUP_W], f32, tag=f"elo{q}", name=f"elo{q}")
                      for q in range(grp // 4)]
            for q in range(grp // 4):
                chain(nc.scalar.activation(ehi_ts[q][:], hi_ts[q][:], ACT.Erf))
                chain(nc.scalar.activation(elo_ts[q][:], lo_ts[q][:], ACT.Erf))
            for j in range(grp):
                s = gi * grp + j
                ps2 = ps2_pool.tile([64, SUP_W], f32, tag="ps2")
                phase2(s, s2gs[j], ehi_ts[j // 4], elo_ts[j // 4], ps2)
                drain_ps2(s, ps2)

        # --- tail ---
        for half in range(2):
            ln_n = tail_pool.tile([32, 128 * n_half], f32, tag="ln_n", name=f"ln_n{half}", bufs=1)
            ln_d = tail_pool.tile([32, 128 * n_half], f32, tag="ln_d", name=f"ln_d{half}", bufs=1)
            chain(nc.scalar.activation(ln_n[:], packed[half][0:32, :], ACT.Ln))
            chain(nc.scalar.activation(ln_d[:], packed[half][32:64, :], ACT.Ln))
            nll = tail_pool.tile([32, 128 * n_half], f32, tag="nll", name=f"nll{half}", bufs=1)
            nc.vector.tensor_tensor(nll[:], ln_d[:], ln_n[:], ALU.subtract)
            nc.sync.dma_start(out[0:12, 128 * n_half * half:128 * n_half * (half + 1)],
                              nll[0:12, :])

    nc.compile()
    return nc


def prep_core_inputs(px_z_shard, x_shard, consts):
    """px_z_shard [nb, 512, 64, 64], x_shard [nb, 64, 64, 3] -> input map."""
    wt, bx, l1, scb = consts
    nb = px_z_shard.shape[0]
    S = nb * (SIZE * SIZE) // SUP_W
    pzs = np.ascontiguousarray(px_z_shard.reshape(nb, WIDTH, SIZE * SIZE))
    xf = x_shard.reshape(S, SUP_W, C_IMG)
    x4 = np.ones((S, 4, SUP_W), np.float32)
    x4[:, 0:3, :] = xf.transpose(0, 2, 1)
    return {
        "pz": pzs, "x4": np.ascontiguousarray(x4), "wt": wt, "bx": bx,
        "l1": l1, "scb": scb,
    }


def gather_core_output(o, nb):
    """o [12, 128*n_sub] (row 4c+g', col sub*128+p') -> [nb, 64, 64, 3]."""
    n_sub = nb * (SIZE * SIZE) // SUB_W
    return (
        o.reshape(C_IMG, 4, n_sub, 128).transpose(2, 1, 3, 0)
        .reshape(nb, SIZE, SIZE, C_IMG)
    )


_NC_CACHE = {}


def kernel(px_z, x, W, b):
    from concourse.bass_utils import run_bass_kernel_spmd

    px_z = np.asarray(px_z, np.float32)
    x = np.asarray(x, np.float32)
    B = px_z.shape[0]
    nb = B // N_CORES
    consts = make_consts(W, b)
    key = (nb,)
    if key not in _NC_CACHE:
        _NC_CACHE[key] = build_nc(n_batch=nb)
    nc = _NC_CACHE[key]
    in_maps = [
        prep_core_inputs(px_z[nb * i:nb * (i + 1)], x[nb * i:nb * (i + 1)], consts)
        for i in range(N_CORES)
    ]
    res = run_bass_kernel_spmd(nc, in_maps, core_ids=list(range(N_CORES)))
    outs = [gather_core_output(res.results[i]["out"], nb) for i in range(N_CORES)]
    return np.concatenate(outs, 0)



# revision 11
# speedup vs baseline: 2.4998x; 2.4998x over previous
"""Discretized-mixture NLL loss kernel for Trainium2 (Bass/Tile), 8-core data parallel.

Math (per pixel, per channel c, mixtures m=0..9), matching the reference:
    xhat = W @ px + b            (1x1 conv, 90 outputs = [pi(30) | mu(30) | ls(30)])
    sigma = exp(8*tanh(ls/8));  s2 = (1/sigma)/sqrt(2) = exp(-8*tanh(ls/8) + ln(1/sqrt2))
    D = mu - xe                  (xe folded into the conv via an extra K=4 matmul)
    dcdf = 0.5*(erf((D+d)*s2) - erf((D-d)*s2))          [erf odd]
    num  = sum_m exp(l_m)*dcdf_m ;  den = sum_m exp(l_m)
    nll  = log(den) - log(num + 1e-8*den)

v2 layout (per core, 16384 px = 16 supertiles of 1024, 2 subtiles of 512 each).
Compute-engine APs must start at partition 0/32/64/96; logical 30-row blocks are
padded to 32.
  - psum tile [128, 1024] (2 banks, pool bufs=3): rows 0..31 D, 32..63 ls,
    64..95 pi, 96..127 tanh-out. fp32r matmuls (full rate when PE warm),
    4 K-chunks of 128 + K=4 chunk carrying (-xe, bias). The psum tile is
    released at the end of phase 1 (mixture reduction uses its own psum pool),
    so the PE pipeline is never serialized behind phase 2.
  - ScalarE: tanh; merged Exp (per-partition scale/bias AP) -> [g | s2].
    Erf over 8-supertile groups (4x [128, 1024] ops) to bound table switches
    (exp-set phase / erf-set phase per group, ~2 ACT_TABLE_LOADs per group).
  - VectorE: fused STT (D -+ delta)*s2 -> hi/lo rows 32*(s%4) of group tiles;
    dlt = Elo - Ehi on GpSimd; qd = g*dlt overwrites dead s2 rows.
  - PE reduction: per subtile one [K=64, M=32, N=512] fp32r matmul into a
    shared psum2 bank ([128, 512]; block base 64*(s%2)+32*t, M=32 cols =
    [n0 n1 n2 X d0 d1 d2 X | 24 dummies], dummies keep Ln inputs finite).
  - ScalarE copy psum2 bank -> scratch; per-subtile respread DMA -> packed
    (row q = 4*v + g', v = L-column 0..15, g' = col/128). Tail per half:
    Ln[64, 2048]; subtract rows 32..63 - 0..31; DMA rows 0..11 out.
"""

import numpy as np

WIDTH = 512
C_IMG = 3
N_MIX = 10
SIZE = 64
STD = 127.5
EPS = 1e-8
DELTA = 1.0 / STD / 2.0
LOG_INV_SQRT2 = -0.34657359027997264
N_CORES = 8
SUP_W = 1024          # pixels per supertile
SUB_W = 512           # matmul moving-dim tile
GRP = 8               # supertiles per activation-table group


def make_consts(W, b):
    """Host-side prep of the small constant tensors (32-padded blocks)."""
    import ml_dtypes

    W = np.asarray(W, np.float32)
    b = np.asarray(b, np.float32)
    # lhsT column blocks: [mu(30)+2 | ls(30)+2 | pi(30)+2]; intra-block row 3m+c
    Wp = np.zeros((96, WIDTH), np.float32)
    bp = np.zeros(96, np.float32)
    Wp[0:30], bp[0:30] = W[30:60], b[30:60]     # mu
    Wp[32:62], bp[32:62] = W[60:90], b[60:90]   # logsigma
    Wp[64:94], bp[64:94] = W[0:30], b[0:30]     # pi logits
    wt = np.ascontiguousarray(Wp.T.astype(ml_dtypes.bfloat16))  # [512, 96] bf16
    bx = np.zeros((4, 96), np.float32)          # K=4 rows: (xe0, xe1, xe2, ones)
    for r in range(30):
        bx[r % 3, r] = -1.0                     # D rows get -xe_c
    bx[3, :] = bp                               # ones row carries the conv bias
    # reduction over s2g = [g(32) | qd(32)]; M=32 columns:
    # v=0..2 num_c, v=3..7 dummy, v=8..10 den_c, v=11..31 dummy (positive)
    l1 = np.zeros((64, 64), np.float32)
    for r in range(30):
        c = r % 3
        l1[r, c] = EPS                          # eps*den folded into num column
        l1[r, 8 + c] = 1.0                      # den
        l1[32 + r, c] = 0.5                     # +0.5*qd -> num
    for v in list(range(3, 8)) + list(range(11, 64)):
        l1[0:30, v] = 1.0                       # dummies = den-like, keep Ln finite
    scb = np.zeros((64, 2), np.float32)         # merged-exp (scale, bias) rows
    scb[0:32, 0] = 1.0                          # pi rows: exp(x)
    scb[32:64, 0] = -8.0                        # tanh rows: exp(-8*t + ln(1/sqrt2))
    scb[32:64, 1] = LOG_INV_SQRT2
    return wt, bx, l1, scb


def build_nc(n_batch=4, use_f32r=True):
    """Build the single-core Bass program (same NEFF runs SPMD on all cores)."""
    from contextlib import ExitStack

    import concourse.bacc as bacc
    import concourse.mybir as mybir
    import concourse.tile as tile
    from concourse.tile import add_dep_helper

    f32 = mybir.dt.float32
    f32r = mybir.dt.float32r
    bf16 = mybir.dt.bfloat16
    ALU = mybir.AluOpType
    ACT = mybir.ActivationFunctionType

    def mm_cast(ap):
        return ap.bitcast(f32r) if use_f32r else ap

    sup_per_batch = (SIZE * SIZE) // SUP_W
    S = n_batch * sup_per_batch                 # supertiles per core
    n_sub = S * (SUP_W // SUB_W)                # total subtiles
    grp = min(GRP, S)
    assert S % grp == 0 and grp % 2 == 0

    nc = bacc.Bacc("TRN2", target_bir_lowering=False, debug=False)
    pz = nc.dram_tensor("pz", [n_batch, WIDTH, SIZE * SIZE], bf16, kind="ExternalInput").ap()
    x4 = nc.dram_tensor("x4", [S, 4, SUP_W], f32, kind="ExternalInput").ap()
    wt = nc.dram_tensor("wt", [WIDTH, 96], bf16, kind="ExternalInput").ap()
    bx = nc.dram_tensor("bx", [4, 96], f32, kind="ExternalInput").ap()
    l1 = nc.dram_tensor("l1", [64, 64], f32, kind="ExternalInput").ap()
    scb = nc.dram_tensor("scb", [64, 2], f32, kind="ExternalInput").ap()
    out = nc.dram_tensor("out", [12, 128 * n_sub], f32, kind="ExternalOutput").ap()

    with tile.TileContext(nc) as tc, ExitStack() as ctx:
        const_pool = ctx.enter_context(tc.tile_pool(name="const", bufs=1))
        xq_pool = ctx.enter_context(tc.tile_pool(name="xq", bufs=2))
        xt_pool = ctx.enter_context(tc.tile_pool(name="xt", bufs=5))
        s2g_pool = ctx.enter_context(tc.tile_pool(name="s2g", bufs=grp + 1))
        hl_pool = ctx.enter_context(tc.tile_pool(name="hl", bufs=2))
        e_pool = ctx.enter_context(tc.tile_pool(name="e", bufs=2))
        dlt_pool = ctx.enter_context(tc.tile_pool(name="dlt", bufs=2))
        sc_pool = ctx.enter_context(tc.tile_pool(name="sc", bufs=2))
        tail_pool = ctx.enter_context(tc.tile_pool(name="tail", bufs=1))
        ps_pool = ctx.enter_context(tc.tile_pool(name="ps", bufs=2, space="PSUM"))
        ps2_pool = ctx.enter_context(tc.tile_pool(name="ps2", bufs=2, space="PSUM"))

        # --- constants ---
        wt_sb = const_pool.tile([128, 4 * 96], bf16)
        nc.sync.dma_start(
            wt_sb[:].rearrange("i (k o) -> i k o", o=96),
            wt.rearrange("(k i) o -> i k o", i=128),
        )
        bx_sb = const_pool.tile([4, 96], f32)
        nc.sync.dma_start(bx_sb[:].bitcast(f32r), bx.bitcast(f32r))
        l1_sb = const_pool.tile([64, 64], f32)
        nc.sync.dma_start(l1_sb[:].bitcast(f32r), l1.bitcast(f32r))
        scb_sb = const_pool.tile([64, 2], f32)
        nc.sync.dma_start(scb_sb[:], scb)

        n_half = n_sub // 2
        packed = [tail_pool.tile([64, 128 * n_half], f32, tag=f"packed{i}", name=f"packed{i}")
                  for i in range(2)]

        # ACT table-set ordering chain (tanh/exp/erf/ln; Copy is in every set)
        act_chain = []

        def chain(inst):
            if act_chain:
                add_dep_helper(inst.ins, act_chain[-1].ins, sync=False,
                               reason="act table-set batching")
            act_chain.append(inst)
            return inst

        def phase1(s, hi_t, lo_t):
            b, h = divmod(s, sup_per_batch)
            hb = 32 * ((s % grp) % 4)
            xq_t = xq_pool.tile([4, SUP_W], f32, tag="xq")
            nc.sync.dma_start(xq_t[:].bitcast(f32r), x4[s].bitcast(f32r))
            # one bf16 supertile slab [128, k=4, 1024], split across the two
            # HWDGE queues (sync + scalar) for DMA-queue parallelism
            xt_t = xt_pool.tile([128, 4 * SUP_W], bf16, tag="xt")
            xt_v = xt_t[:].rearrange("i (k n) -> i k n", n=SUP_W)
            pz_v = pz[b, :, SUP_W * h:SUP_W * (h + 1)].rearrange(
                "(k i) n -> i k n", i=128)
            nc.sync.dma_start(xt_v[:, 0:2], pz_v[:, 0:2])
            nc.scalar.dma_start(xt_v[:, 2:4], pz_v[:, 2:4])
            ps = ps_pool.tile([128, SUP_W], f32, tag="ps")
            n_t = SUP_W // SUB_W
            for k in range(4):
                for t in range(n_t):
                    sl = slice(SUB_W * t, SUB_W * (t + 1))
                    nc.tensor.matmul(
                        ps[0:96, sl], wt_sb[:, 96 * k:96 * (k + 1)],
                        xt_v[:, k, sl], start=(k == 0), stop=False,
                    )
            for t in range(n_t):
                sl = slice(SUB_W * t, SUB_W * (t + 1))
                nc.tensor.matmul(ps[0:96, sl], mm_cast(bx_sb[:]),
                                 mm_cast(xq_t[:, sl]), start=False, stop=True)
            s2g_t = s2g_pool.tile([64, SUP_W], f32, tag="s2g")
            chain(nc.scalar.activation(ps[96:128, :], ps[32:64, :], ACT.Tanh, scale=0.125))
            chain(nc.scalar.activation(
                s2g_t[:].bitcast(f32r), ps[64:128, :], ACT.Exp,
                bias=scb_sb[:, 1:2], scale=scb_sb[:, 0:1],
            ))
            # hi' = (D - delta)*s2 ; lo' = (D + delta)*s2
            nc.vector.scalar_tensor_tensor(
                hi_t[hb:hb + 32, :], ps[0:32, :], DELTA,
                s2g_t[32:64, :], ALU.subtract, ALU.mult,
            )
            nc.vector.scalar_tensor_tensor(
                lo_t[hb:hb + 32, :], ps[0:32, :], DELTA,
                s2g_t[32:64, :], ALU.add, ALU.mult,
            )
            return s2g_t

        def phase2(s, s2g_t, ehi_t, elo_t, ps2):
            hb = 32 * ((s % grp) % 4)
            g = s2g_t[0:32, :]
            dlt_t = dlt_pool.tile([32, SUP_W], f32, tag="dlt")
            nc.gpsimd.tensor_tensor(dlt_t[:], elo_t[hb:hb + 32, :],
                                    ehi_t[hb:hb + 32, :], ALU.subtract)
            # qd = g*dlt overwrites the dead s2 rows -> [g | qd] contiguous
            nc.vector.tensor_tensor(s2g_t[32:64, :].bitcast(f32r), g, dlt_t[:], ALU.mult)
            for t in range(SUP_W // SUB_W):
                sl = slice(SUB_W * t, SUB_W * (t + 1))
                nc.tensor.matmul(ps2[0:64, sl], mm_cast(l1_sb[:]),
                                 mm_cast(s2g_t[:, sl]), start=True, stop=True)

        def drain_ps2(s, ps2):
            # copy the psum2 banks, then respread each subtile block to packed
            sc_t = sc_pool.tile([64, SUP_W], f32, tag="sc")
            nc.scalar.copy(sc_t[:], ps2[:])
            for t in range(SUP_W // SUB_W):
                sub = s * (SUP_W // SUB_W) + t
                half, subh = divmod(sub, n_half)
                nc.sync.dma_start(
                    packed[half][:, 128 * subh:128 * (subh + 1)],
                    sc_t[0:16, SUB_W * t:SUB_W * (t + 1)]
                    .rearrange("v (g p) -> v g p", p=128),
                )

        n_grp = S // grp
        for gi in range(n_grp):
            hi_ts = [hl_pool.tile([128, SUP_W], f32, tag=f"hi{q}", name=f"hi{q}")
                     for q in range(grp // 4)]
            lo_ts = [hl_pool.tile([128, SUP_W], f32, tag=f"lo{q}", name=f"lo{q}")
                     for q in range(grp // 4)]
            s2gs = []
            for j in range(grp):
                s = gi * grp + j
                s2gs.append(phase1(s, hi_ts[j // 4], lo_ts[j // 4]))
            ehi_ts = [e_pool.tile([128, SUP_W], f32, tag=f"ehi{q}", name=f"ehi{q}")
                      for q in range(grp // 4)]
            elo_ts = [e_pool.tile([128, SUP_W], f32, tag=f"elo{q}", name=f"elo{q}")
                      for q in range(grp // 4)]
            for q in range(grp // 4):
                chain(nc.scalar.activation(ehi_ts[q][:], hi_ts[q][:], ACT.Erf))
                chain(nc.scalar.activation(elo_ts[q][:], lo_ts[q][:], ACT.Erf))
            for j in range(grp):
                s = gi * grp + j
                ps2 = ps2_pool.tile([64, SUP_W], f32, tag="ps2")
                phase2(s, s2gs[j], ehi_ts[j // 4], elo_ts[j // 4], ps2)
                drain_ps2(s, ps2)

        # --- tail ---
        for half in range(2):
            ln_n = tail_pool.tile([32, 128 * n_half], f32, tag="ln_n", name=f"ln_n{half}", bufs=1)
            ln_d = tail_pool.tile([32, 128 * n_half], f32, tag="ln_d", name=f"ln_d{half}", bufs=1)
            chain(nc.scalar.activation(ln_n[:], packed[half][0:32, :], ACT.Ln))
            chain(nc.scalar.activation(ln_d[:], packed[half][32:64, :], ACT.Ln))
            nll = tail_pool.tile([32, 128 * n_half], f32, tag="nll", name=f"nll{half}", bufs=1)
            nc.vector.tensor_tensor(nll[:], ln_d[:], ln_n[:], ALU.subtract)
            nc.sync.dma_start(out[0:12, 128 * n_half * half:128 * n_half * (half + 1)],
                              nll[0:12, :])

    nc.compile()
    return nc


def prep_core_inputs(px_z_shard, x_shard, consts):
    """px_z_shard [nb, 512, 64, 64], x_shard [nb, 64, 64, 3] -> input map."""
    import ml_dtypes

    wt, bx, l1, scb = consts
    nb = px_z_shard.shape[0]
    S = nb * (SIZE * SIZE) // SUP_W
    pzs = np.ascontiguousarray(
        px_z_shard.reshape(nb, WIDTH, SIZE * SIZE).astype(ml_dtypes.bfloat16))
    xf = x_shard.reshape(S, SUP_W, C_IMG)
    x4 = np.ones((S, 4, SUP_W), np.float32)
    x4[:, 0:3, :] = xf.transpose(0, 2, 1)
    return {
        "pz": pzs, "x4": np.ascontiguousarray(x4), "wt": wt, "bx": bx,
        "l1": l1, "scb": scb,
    }


def gather_core_output(o, nb):
    """o [12, 128*n_sub] (row 4c+g', col sub*128+p') -> [nb, 64, 64, 3]."""
    n_sub = nb * (SIZE * SIZE) // SUB_W
    return (
        o.reshape(C_IMG, 4, n_sub, 128).transpose(2, 1, 3, 0)
        .reshape(nb, SIZE, SIZE, C_IMG)
    )


_NC_CACHE = {}


def kernel(px_z, x, W, b):
    from concourse.bass_utils import run_bass_kernel_spmd

    px_z = np.asarray(px_z, np.float32)
    x = np.asarray(x, np.float32)
    B = px_z.shape[0]
    nb = B // N_CORES
    consts = make_consts(W, b)
    key = (nb,)
    if key not in _NC_CACHE:
        _NC_CACHE[key] = build_nc(n_batch=nb)
    nc = _NC_CACHE[key]
    in_maps = [
        prep_core_inputs(px_z[nb * i:nb * (i + 1)], x[nb * i:nb * (i + 1)], consts)
        for i in range(N_CORES)
    ]
    res = run_bass_kernel_spmd(nc, in_maps, core_ids=list(range(N_CORES)))
    outs = [gather_core_output(res.results[i]["out"], nb) for i in range(N_CORES)]
    return np.concatenate(outs, 0)

